# revision 32
# baseline (speedup 1.0000x reference)
"""GQA sparse-attention kernel for 8 Trainium2 NeuronCores.

Sharding: data-parallel over batch (2) x sequence-parallel over query rows
(4 row-groups per batch, rows j::4 interleaved so causal work is balanced and
the program is SPMD-identical across cores). No collectives: each core holds
512 query rows and computes all 16 heads for them, then layernorm + out-proj
for its rows locally.

v2 restructure vs the first working version:
  - causal trapezoid at 32-column granularity: for s-block k only query
    columns [32k, 512) are computed (4352 cols/head vs 6144 dense-ish),
    cutting sim/PV matmul columns, exp and mask-multiply work by ~30%.
  - sim results for complementary s-block pairs (k, 16-k) pack exactly into
    512-col PSUM banks; 3 chunk tiles [128,1536] per head -> only 3 ACT exp
    calls per head (the ~352-cycle ACT per-call overhead dominates small
    tiles).
  - masks are pre-packed on host as f16 (causal & adj) in the exact chunk
    layout -> 1.1MB DMA instead of 4MB int32 + on-device conversion.
  - kv-head parity pairs (even h at partitions 0-63, odd at 64-127) are
    interleaved so the K=64 sim matmuls run on PE row-tiles T0/T8
    concurrently and exp/mask of one head overlaps sim of the other.
  - PV weights are 65 wide (64 v dims + ones column for the denominator).
  - normalization/LN broadcasts go through gpsimd.partition_broadcast
    instead of PE ones-matmuls (frees PSUM banks + PE cycles).
  - LN applies in f16 (DVE 2x mode).
"""

import os
import sys

import numpy as np

for _p in ("/opt/trn_rl_repo", "/root/.axon_site/_ro/trn_rl_repo"):
    if os.path.isdir(_p) and _p not in sys.path:
        sys.path.insert(0, _p)

B, N, E = 2, 2048, 1024
HQ, HK, D = 16, 4, 64
G = HQ // HK          # 4 query heads per kv head
KVE = HK * D          # 256
NL = N // 4           # 512 local query rows per core
SB = N // 128         # 16 s-blocks
EC = E // 128         # 8 embedding chunks
LN_EPS = 1e-5

# trapezoid: for s-block k, query cols [32k, 512) are live
def _cols(k):
    return 512 - 32 * k

# chunk layout: complementary s-block pairs (j, 16-j) sum to exactly 512
# cols, so each chunk packs into full PSUM banks with no padding.
CH = [
    [(0, 0), (1, 512), (15, 992), (2, 1024), (14, 1472)],
    [(3, 0), (13, 416), (4, 512), (12, 896), (5, 1024), (11, 1376)],
    [(6, 0), (10, 320), (7, 512), (9, 800), (8, 1024)],
]
CHW = [1536, 1536, 1280]
CHBASE = [0, 1536, 3072]
MASKW = 4352
PIPE = 1   # chunks of sim/exp lookahead emitted before the matching PV

_PROG_CACHE = {}


def build_program(with_em=True):
    import concourse.bass as bass
    import concourse.mybir as mybir
    import concourse.tile as tile
    from concourse import bacc

    dt = mybir.dt
    f32, bf16, i32 = dt.float32, dt.bfloat16, dt.int32
    f32r = dt.float32r
    f16 = dt.float16
    AF = mybir.ActivationFunctionType
    OP = mybir.AluOpType
    AX = mybir.AxisListType

    nc = bacc.Bacc("TRN2", target_bir_lowering=False, debug=False)

    def din(name, shape, dtp=f32):
        return nc.dram_tensor(name, shape, dtp, kind="ExternalInput").ap()

    xqT = din("xqT", [E, NL], f16)
    xkT = din("xkT", [E, N], f16)
    xvT = din("xvT", [E, N], f16)
    adjm = din("adjm", [128, MASKW], f16)   # host-packed chunk-layout mask
    WqT = din("WqT", [E, E], f16)           # pre-scaled by 1/8 on host
    WkT = din("WkT", [E, KVE], f16)
    WvT = din("WvT", [E, KVE], f16)
    WoT = din("WoT", [E, E], f16)
    bq2d = din("bq2d", [EC, 128])           # bq/8
    bk2d = din("bk2d", [2, 128])
    bv2d = din("bv2d", [2, 128])
    bo1 = din("bo1", [1, E], f16)
    lng = din("lng", [EC, 128])
    lnb = din("lnb", [EC, 128])
    ones1h = din("ones1h", [1, 128], f16)
    y = nc.dram_tensor("y", [NL, E], f32, kind="ExternalOutput").ap()

    with tile.TileContext(nc) as tc, nc.allow_low_precision(
            "f16 operands for PE fast-path matmuls are intentional"):
        with (
            tc.tile_pool(name="const", bufs=1) as pc,
            tc.tile_pool(name="persist", bufs=1) as pp,
        ):
            ident = pc.tile([128, 128], f16, tag="ident")
            from concourse.masks import make_identity
            make_identity(nc, ident[:])
            ones_m1 = pc.tile([128, 1], f16, tag="ones_m1")
            nc.sync.dma_start(ones_m1[:], ones1h)
            ones_k1h = pc.tile([1, 128], f16, tag="ones_k1h")
            nc.sync.dma_start(ones_k1h[:], ones1h)
            eps_c = pc.tile([1, 1], f32, tag="eps_c")
            nc.gpsimd.memset(eps_c[:], LN_EPS)

            # persistent tiles (out-proj/LN constants: DMAs emitted late, after
            # phase 2, so they don't block the queue ahead of phase-1 inputs)
            wo_sb = [pp.tile([128, E], f16, tag=f"wo{e}", name=f"wo{e}") for e in range(EC)]
            bo_sb = pp.tile([1, E], f16, tag="bo", name="bo")
            lng_sb = [pp.tile([128, 1], f32, tag=f"lng{e}", name=f"lng{e}") for e in range(EC)]
            lnb_sb = [pp.tile([128, 1], f32, tag=f"lnb{e}", name=f"lnb{e}") for e in range(EC)]

            kT_sb = [pp.tile([128, N], f16, tag=f"kt{m}", name=f"kt{m}") for m in range(2)]
            # v_ext[k]: [128 s, 4 heads x 65] = per head 64 v dims + ones col
            v_ext = [pp.tile([128, 4 * 65], f16, tag=f"vx{k}", name=f"vx{k}") for k in range(SB)]
            meanv = [pp.tile([128, 1], f32, tag=f"mv{m}", name=f"mv{m}") for m in range(2)]
            mv2 = [pp.tile([128, 1], f32, tag=f"mv2{h}", name=f"mv2{h}") for h in range(HK)]
            # per-head q tiles, zero-padded to the full 128 partitions: head g
            # (kv head h = g//4) holds data at partitions [(h%2)*64, +64) --
            # matching h's partition half inside kT_sb[h//2] -- and ZEROS in
            # the other half. The sim matmul then contracts over all 128
            # partitions of kT_sb (both kv heads), with the zero half of q
            # nulling the wrong head's contribution. K=128 keeps the PE in
            # full-array mode at full streaming rate (a solo K=64 stream is
            # hardware-limited to half rate).
            qz = [pp.tile([128, NL], f16, tag=f"qz{g}", name=f"qz{g}") for g in range(HQ)]
            for g in range(HQ):
                pad = (1 - (g // G) % 2) * 64
                nc.gpsimd.memset(qz[g][pad:pad + 64, :], 0.0)
            attT = [pp.tile([128, NL], f16, tag=f"at{e}", name=f"at{e}") for e in range(EC)]
            mk_all = pp.tile([128, MASKW], f16, tag="mk", name="mk")
            if with_em:
                em_row = pp.tile([1, NL], f32, tag="em_row", name="em_row")
                em_bc = pp.tile([128, NL], f32, tag="em_bc", name="em_bc")

            # ---------------- phase 1: q projection ----------------
            with (
                tc.tile_pool(name="wq", bufs=1) as pwq,
                tc.tile_pool(name="xq", bufs=1) as pxq,
                tc.tile_pool(name="bq", bufs=1) as pbq,
                tc.tile_pool(name="psq", bufs=2, space="PSUM") as psq,
            ):
                wq_sb = [pwq.tile([128, E], f16, tag=f"wq{e}", name=f"wq{e}") for e in range(EC)]
                xq_sb = [pxq.tile([128, NL], f16, tag=f"xq{e}", name=f"xq{e}") for e in range(EC)]
                bq_sb = [pbq.tile([128, 1], f32, tag=f"bq{m}", name=f"bq{m}") for m in range(EC)]
                for e in range(EC):
                    nc.sync.dma_start(wq_sb[e][:], WqT[e * 128:(e + 1) * 128, :])
                    nc.sync.dma_start(xq_sb[e][:], xqT[e * 128:(e + 1) * 128, :])
                    nc.sync.dma_start(bq_sb[e][:], bq2d[e:e + 1, :])
                nc.sync.dma_start(mk_all[:], adjm)
                for mt2 in range(EC // 2):
                    psA = psq.tile([128, NL], f32, tag="psqA", name="psqA")
                    psB = psq.tile([128, NL], f32, tag="psqB", name="psqB")
                    for e in range(EC):
                        for mt, ps in ((2 * mt2, psA), (2 * mt2 + 1, psB)):
                            nc.tensor.matmul(
                                ps[:], wq_sb[e][:, mt * 128:(mt + 1) * 128],
                                xq_sb[e][:], start=(e == 0), stop=(e == EC - 1))
                    for mt, ps in ((2 * mt2, psA), (2 * mt2 + 1, psB)):
                     for t in range(2):
                        g = 2 * mt + t
                        slot = (g // G) % 2
                        nc.scalar.activation(
                            qz[g][slot * 64:(slot + 1) * 64, :],
                            ps[t * 64:(t + 1) * 64, :], AF.Identity,
                            bias=bq_sb[mt][t * 64:(t + 1) * 64, :], scale=1.0)

            # ---------------- phase 2: k/v projections ----------------
            with (
                tc.tile_pool(name="wkv", bufs=1) as pwkv,
                tc.tile_pool(name="xkv", bufs=3) as pxkv,
                tc.tile_pool(name="vt", bufs=2) as pvt,
                tc.tile_pool(name="vs", bufs=1) as pvs,
                tc.tile_pool(name="pskv", bufs=2, space="PSUM") as pskv,
                tc.tile_pool(name="pst", bufs=2, space="PSUM") as pst,
            ):
                wk_sb = [pwkv.tile([128, KVE], f16, tag=f"wk{e}", name=f"wk{e}") for e in range(EC)]
                wv_sb = [pwkv.tile([128, KVE], f16, tag=f"wv{e}", name=f"wv{e}") for e in range(EC)]
                bk_sb = [pwkv.tile([128, 1], f32, tag=f"bk{m}", name=f"bk{m}") for m in range(2)]
                bv_sb = [pwkv.tile([128, 1], f32, tag=f"bv{m}", name=f"bv{m}") for m in range(2)]
                for e in range(EC):
                    nc.sync.dma_start(wk_sb[e][:], WkT[e * 128:(e + 1) * 128, :])
                    nc.sync.dma_start(wv_sb[e][:], WvT[e * 128:(e + 1) * 128, :])
                for m in range(2):
                    nc.sync.dma_start(bk_sb[m][:], bk2d[m:m + 1, :])
                    nc.sync.dma_start(bv_sb[m][:], bv2d[m:m + 1, :])
                vsum = [pvs.tile([128, 4], f32, tag=f"vsum{m}", name=f"vsum{m}") for m in range(2)]
                for st in (0, 3, 1, 2):  # chunk 0 needs s-tiles 0 and 3 first
                    sl = slice(st * 512, (st + 1) * 512)
                    xk_sb = [pxkv.tile([128, 512], f16, tag=f"xk{e % 2}", name=f"xk{e}") for e in range(EC)]
                    xv_sb = [pxkv.tile([128, 512], f16, tag=f"xv{e % 2}", name=f"xv{e}") for e in range(EC)]
                    for e in range(EC):
                        nc.sync.dma_start(xk_sb[e][:], xkT[e * 128:(e + 1) * 128, sl])
                        nc.sync.dma_start(xv_sb[e][:], xvT[e * 128:(e + 1) * 128, sl])
                    for mt in range(2):
                        psk = pskv.tile([128, 512], f32, tag="psk")
                        psv = pskv.tile([128, 512], f32, tag="psv")
                        for e in range(EC):
                            nc.tensor.matmul(
                                psk[:], wk_sb[e][:, mt * 128:(mt + 1) * 128],
                                xk_sb[e][:], start=(e == 0), stop=(e == EC - 1))
                            nc.tensor.matmul(
                                psv[:], wv_sb[e][:, mt * 128:(mt + 1) * 128],
                                xv_sb[e][:], start=(e == 0), stop=(e == EC - 1))
                        nc.scalar.activation(kT_sb[mt][:, sl], psk[:], AF.Identity,
                                             bias=bk_sb[mt][:], scale=1.0)
                        vt = pvt.tile([128, 512], f16, tag="vt")
                        nc.scalar.activation(vt[:], psv[:], AF.Identity,
                                             bias=bv_sb[mt][:], scale=1.0)
                        nc.vector.reduce_sum(vsum[mt][:, st:st + 1], vt[:], axis=AX.X)
                        for ss in range(4):
                            k = st * 4 + ss
                            pt = pst.tile([128, 128], f16, tag="pt")
                            nc.tensor.transpose(pt[:], vt[:, ss * 128:(ss + 1) * 128],
                                                ident[:])
                            src = pt[:].rearrange("p (h x) -> p h x", h=2)
                            dst = v_ext[k][:].rearrange("p (h x) -> p h x", h=4)
                            nc.vector.tensor_copy(dst[:, 2 * mt:2 * mt + 2, 0:64], src)
                for k in range(SB):
                    one_col = v_ext[k][:].rearrange("p (h x) -> p h x", h=4)[:, :, 64:65]
                    nc.gpsimd.memset(one_col, 1.0)
                for m in range(2):
                    nc.vector.tensor_reduce(meanv[m][:], vsum[m][:], axis=AX.X,
                                            op=OP.add)
                    nc.vector.tensor_scalar_mul(meanv[m][:], meanv[m][:], 1.0 / N)
                for h in range(HK):
                    src = meanv[h // 2][(h % 2) * 64:(h % 2) * 64 + 64, :]
                    nc.vector.tensor_copy(mv2[h][0:64, :], src)
                    nc.vector.tensor_copy(mv2[h][64:128, :], src)

            # late DMAs: out-proj/LN constants, needed only in phase 4
            for e in range(EC):
                nc.sync.dma_start(wo_sb[e][:], WoT[e * 128:(e + 1) * 128, :])
                nc.sync.dma_start(lng_sb[e][:], lng[e:e + 1, :])
                nc.sync.dma_start(lnb_sb[e][:], lnb[e:e + 1, :])
            nc.sync.dma_start(bo_sb[:], bo1)

            # ---------------- phase 3: attention ----------------
            # empty (fully-masked) rows can only be tiny global row indices
            # (P ~ 2^-(r+1)); they land in local columns < EMW, so the
            # empty-row fallback ops are restricted to that column range.
            EMW = NL
            with (
                tc.tile_pool(name="exs", bufs=3) as pex,
                tc.tile_pool(name="nrm", bufs=2) as pnrm,
                tc.tile_pool(name="pssim", bufs=1, space="PSUM") as psim,
                tc.tile_pool(name="pspv", bufs=1, space="PSUM") as ppv,
            ):
                first = True
                for pair in range(2):
                    for gi in range(G):
                        heads = [(2 * pair, 0), (2 * pair + 1, 1)]
                        pvs_t = {}
                        for h, slot in heads:
                            pvs_t[slot] = ppv.tile([128, 512], f32, tag=f"pv{slot}",
                                                   name=f"pv{slot}")

                        def emit_pv(blocks, exs):
                            for h, slot in heads:
                                ex = exs[slot]
                                for k, off in blocks:
                                    c = _cols(k)
                                    nc.tensor.matmul(
                                        pvs_t[slot][0:65, 32 * k:512],
                                        v_ext[k][:, 65 * h:65 * h + 65],
                                        ex[:, off:off + c],
                                        start=(k == 0), stop=(k == 8),
                                        skip_group_check=True)

                        pend = []
                        for ci, blocks in enumerate(CH):
                            W = CHW[ci]
                            exs = {}
                            sims = {}
                            for h, slot in heads:
                                sims[slot] = psim.tile([128, 1536], f32,
                                                       tag=f"sim{slot}",
                                                       name=f"sim{slot}")
                            for h, slot in heads:
                                g = h * G + gi
                                kh = kT_sb[h // 2]
                                for k, off in blocks:
                                    c = _cols(k)
                                    nc.tensor.matmul(
                                        sims[slot][:, off:off + c],
                                        kh[:, k * 128:(k + 1) * 128],
                                        qz[g][:, 32 * k:512],
                                        start=True, stop=True)
                            for h, slot in heads:
                                ex = pex.tile([128, 1536], f16, tag=f"ex{slot}",
                                              name=f"ex{slot}")
                                nc.scalar.activation(ex[:, 0:W], sims[slot][:, 0:W],
                                                     AF.Exp)
                                # mask multiply: chunk 2 goes to the otherwise
                                # idle GpSimd engine to unload the DVE
                                meng = nc.gpsimd if ci == 2 else nc.vector
                                meng.tensor_tensor(
                                    ex[:, 0:W], ex[:, 0:W],
                                    mk_all[:, CHBASE[ci]:CHBASE[ci] + W], op=OP.mult)
                                exs[slot] = ex
                            pend.append((blocks, exs))
                            if len(pend) > PIPE:
                                emit_pv(*pend.pop(0))
                        while pend:
                            emit_pv(*pend.pop(0))
                        for h, slot in heads:
                            g = h * G + gi
                            pv = pvs_t[slot]
                            if first and with_em:
                                # row-emptiness is head-independent
                                nc.vector.tensor_scalar(em_row[:, 0:EMW],
                                                        pv[64:65, 0:EMW], 0.0,
                                                        None, op0=OP.is_equal)
                                nc.gpsimd.partition_broadcast(em_bc[:, 0:EMW],
                                                              em_row[:, 0:EMW])
                                first = False
                            den = pnrm.tile([1, NL], f32, tag=f"den{slot}",
                                            name=f"den{slot}")
                            if with_em:
                                nc.vector.tensor_tensor(den[:, 0:EMW],
                                                        pv[64:65, 0:EMW],
                                                        em_row[:, 0:EMW], op=OP.add)
                                nc.vector.reciprocal_approx_fast(
                                    den[:, 0:EMW], den[:, 0:EMW])
                                if EMW < NL:
                                    nc.vector.reciprocal_approx_fast(
                                        den[:, EMW:], pv[64:65, EMW:])
                            else:
                                nc.vector.reciprocal_approx_fast(den[:],
                                                                 pv[64:65, :])
                            recb = pnrm.tile([128, NL], f32, tag=f"recb{slot}",
                                             name=f"recb{slot}")
                            nc.gpsimd.partition_broadcast(recb[:], den[:])
                            p0 = (g % 2) * 64
                            att = attT[g // 2][p0:p0 + 64, :]
                            nc.vector.tensor_tensor(att, pv[0:64, :], recb[0:64, :],
                                                    op=OP.mult)
                            if with_em:
                                nc.vector.scalar_tensor_tensor(
                                    att[:, 0:EMW], em_bc[p0:p0 + 64, 0:EMW],
                                    mv2[h][p0:p0 + 64, :], att[:, 0:EMW],
                                    op0=OP.mult, op1=OP.add)

            # ---------------- phase 4: layernorm + out projection ----------------
            with (
                tc.tile_pool(name="lnt", bufs=2) as plnt,
                tc.tile_pool(name="ysb", bufs=2) as pysb,
                tc.tile_pool(name="psst", bufs=1, space="PSUM") as psst,
                tc.tile_pool(name="psy", bufs=2, space="PSUM") as psy,
            ):
                st_sum = psst.tile([1, NL], f32, tag="ssum")
                st_sq = psst.tile([1, NL], f32, tag="ssq")
                for e in range(EC):
                    nc.tensor.matmul(st_sum[:], ones_m1[:], attT[e][:],
                                     start=(e == 0), stop=(e == EC - 1))
                    sq = plnt.tile([128, NL], f16, tag="sq")
                    nc.scalar.activation(sq[:], attT[e][:], AF.Square)
                    nc.tensor.matmul(st_sq[:], ones_m1[:], sq[:],
                                     start=(e == 0), stop=(e == EC - 1))
                mu = plnt.tile([1, NL], f32, tag="mu")
                nc.vector.tensor_scalar_mul(mu[:], st_sum[:], 1.0 / E)
                var = plnt.tile([1, NL], f32, tag="var")
                nc.vector.tensor_scalar_mul(var[:], st_sq[:], 1.0 / E)
                mu2 = plnt.tile([1, NL], f32, tag="mu2")
                nc.vector.tensor_tensor(mu2[:], mu[:], mu[:], op=OP.mult)
                nc.vector.tensor_tensor(var[:], var[:], mu2[:], op=OP.subtract)
                sd = plnt.tile([1, NL], f32, tag="sd")
                nc.scalar.activation(sd[:], var[:], AF.Sqrt, bias=eps_c[:])
                nc.vector.reciprocal(sd[:], sd[:])
                mu_h = plnt.tile([1, NL], f16, tag="mu_h")
                nc.vector.tensor_copy(mu_h[:], mu[:])
                sd_h = plnt.tile([1, NL], f16, tag="sd_h")
                nc.vector.tensor_copy(sd_h[:], sd[:])
                mbb = plnt.tile([128, NL], f16, tag="mbb")
                nc.gpsimd.partition_broadcast(mbb[:], mu_h[:])
                ibb = plnt.tile([128, NL], f16, tag="ibb")
                nc.gpsimd.partition_broadcast(ibb[:], sd_h[:])
                for e in range(EC):
                    nc.vector.tensor_tensor(attT[e][:], attT[e][:], mbb[:],
                                            op=OP.subtract)
                    nc.vector.tensor_tensor(attT[e][:], attT[e][:], ibb[:],
                                            op=OP.mult)
                    nc.vector.tensor_scalar(attT[e][:], attT[e][:], lng_sb[e][:],
                                            lnb_sb[e][:], op0=OP.mult, op1=OP.add)
                for nt in range(4):
                    pyA = psy.tile([128, 512], f32, tag="pyA", name="pyA")
                    pyB = psy.tile([128, 512], f32, tag="pyB", name="pyB")
                    pys = (pyA, pyB)
                    for e in range(EC):
                        for oc in range(2):
                            nc.tensor.matmul(
                                pys[oc][:], attT[e][:, nt * 128:(nt + 1) * 128],
                                wo_sb[e][:, oc * 512:(oc + 1) * 512],
                                start=(e == 0), stop=False)
                    for oc in range(2):
                        nc.tensor.matmul(pys[oc][:], ones_k1h[:],
                                         bo_sb[0:1, oc * 512:(oc + 1) * 512],
                                         start=False, stop=True)
                        ys = pysb.tile([128, 512], f32, tag="ys")
                        nc.scalar.activation(ys[:], pys[oc][:], AF.Identity)
                        nc.sync.dma_start(
                            y[nt * 128:(nt + 1) * 128, oc * 512:(oc + 1) * 512],
                            ys[:])
    nc.finalize()
    return nc


def shard_inputs(inputs):
    q = np.asarray(inputs["query"], np.float32)
    k = np.asarray(inputs["key"], np.float32)
    v = np.asarray(inputs["value"], np.float32)
    adj = np.asarray(inputs["adj"], np.int32)
    WqT8 = (np.ascontiguousarray(np.asarray(inputs["Wq"], np.float32).T)
            / np.float32(8.0)).astype(np.float16)
    WkT = np.ascontiguousarray(np.asarray(inputs["Wk"], np.float32).T).astype(np.float16)
    WvT = np.ascontiguousarray(np.asarray(inputs["Wv"], np.float32).T).astype(np.float16)
    WoT = np.ascontiguousarray(np.asarray(inputs["Wo"], np.float32).T).astype(np.float16)
    bq8 = (np.asarray(inputs["bq"], np.float32) / np.float32(8.0)).reshape(EC, 128)
    bk2 = np.asarray(inputs["bk"], np.float32).reshape(2, 128)
    bv2 = np.asarray(inputs["bv"], np.float32).reshape(2, 128)
    bo1 = np.asarray(inputs["bo"], np.float32).reshape(1, E).astype(np.float16)
    lng = np.asarray(inputs["ln_g"], np.float32).reshape(EC, 128)
    lnb = np.asarray(inputs["ln_b"], np.float32).reshape(EC, 128)

    shared = dict(WqT=WqT8, WkT=WkT, WvT=WvT, WoT=WoT, bq2d=bq8, bk2d=bk2,
                  bv2d=bv2, bo1=bo1, lng=lng, lnb=lnb,
                  ones1h=np.ones((1, 128), np.float16))
    per_b = []
    s_idx = np.arange(N)
    for b in range(B):
        per_b.append((np.ascontiguousarray(k[b].T).astype(np.float16),
                      np.ascontiguousarray(v[b].T).astype(np.float16)))
    in_maps = []
    for c in range(8):
        b, j = divmod(c, 4)
        rows = np.arange(j, N, 4)
        causal = s_idx[None, :] <= rows[:, None]          # [NL, N]
        adjc = np.where(causal, adj[b][rows], 0)          # [NL rows, N s]
        mT = adjc.T.astype(np.float16)                    # [N s, NL]
        adjmp = np.zeros((128, MASKW), np.float16)
        for ci, blocks in enumerate(CH):
            base = CHBASE[ci]
            for kk, off in blocks:
                c_ = 512 - 32 * kk
                adjmp[:, base + off:base + off + c_] = \
                    mT[128 * kk:128 * (kk + 1), 32 * kk:512]
        m = dict(shared)
        m["xqT"] = np.ascontiguousarray(q[b][rows].T).astype(np.float16)
        m["xkT"], m["xvT"] = per_b[b]
        m["adjm"] = adjmp
        in_maps.append(m)
    return in_maps


def unshard_outputs(results):
    out = np.empty((B, N, E), np.float32)
    for c in range(8):
        b, j = divmod(c, 4)
        out[b, j::4, :] = results[c]["y"]
    return out


def kernel(**inputs):
    from concourse.bass_utils import run_bass_kernel_spmd

    if "nc" not in _PROG_CACHE:
        _PROG_CACHE["nc"] = build_program()
    nc = _PROG_CACHE["nc"]
    in_maps = shard_inputs(inputs)
    res = run_bass_kernel_spmd(nc, in_maps, core_ids=list(range(8)))
    return unshard_outputs(res.results)


# revision 33
# speedup vs baseline: 1.3889x; 1.3889x over previous
"""GQA sparse-attention kernel for 8 Trainium2 NeuronCores.

Sharding: data-parallel over batch (2) x sequence-parallel over query rows
(4 row-groups per batch, rows j::4 interleaved so causal work is balanced and
the program is SPMD-identical across cores). No collectives: each core holds
512 query rows and computes all 16 heads for them, then layernorm + out-proj
for its rows locally.

v2 restructure vs the first working version:
  - causal trapezoid at 32-column granularity: for s-block k only query
    columns [32k, 512) are computed (4352 cols/head vs 6144 dense-ish),
    cutting sim/PV matmul columns, exp and mask-multiply work by ~30%.
  - sim results for complementary s-block pairs (k, 16-k) pack exactly into
    512-col PSUM banks; 3 chunk tiles [128,1536] per head -> only 3 ACT exp
    calls per head (the ~352-cycle ACT per-call overhead dominates small
    tiles).
  - masks are pre-packed on host as f16 (causal & adj) in the exact chunk
    layout -> 1.1MB DMA instead of 4MB int32 + on-device conversion.
  - kv-head parity pairs (even h at partitions 0-63, odd at 64-127) are
    interleaved so the K=64 sim matmuls run on PE row-tiles T0/T8
    concurrently and exp/mask of one head overlaps sim of the other.
  - PV weights are 65 wide (64 v dims + ones column for the denominator).
  - normalization/LN broadcasts go through gpsimd.partition_broadcast
    instead of PE ones-matmuls (frees PSUM banks + PE cycles).
  - LN applies in f16 (DVE 2x mode).
"""

import os
import sys

import numpy as np

for _p in ("/opt/trn_rl_repo", "/root/.axon_site/_ro/trn_rl_repo"):
    if os.path.isdir(_p) and _p not in sys.path:
        sys.path.insert(0, _p)

B, N, E = 2, 2048, 1024
HQ, HK, D = 16, 4, 64
G = HQ // HK          # 4 query heads per kv head
KVE = HK * D          # 256
NL = N // 4           # 512 local query rows per core
SB = N // 128         # 16 s-blocks
EC = E // 128         # 8 embedding chunks
LN_EPS = 1e-5

# trapezoid: for s-block k, query cols [32k, 512) are live
def _cols(k):
    return 512 - 32 * k

# chunk layout: complementary s-block pairs (j, 16-j) sum to exactly 512
# cols, so each chunk packs into full PSUM banks with no padding.
CH = [
    [(0, 0), (1, 512), (15, 992), (2, 1024), (14, 1472)],
    [(3, 0), (13, 416), (4, 512), (12, 896), (5, 1024), (11, 1376)],
    [(6, 0), (10, 320), (7, 512), (9, 800), (8, 1024)],
]
CHW = [1536, 1536, 1280]
CHBASE = [0, 1536, 3072]
MASKW = 4352
PIPE = 1   # chunks of sim/exp lookahead emitted before the matching PV

_PROG_CACHE = {}


def build_program(with_em=True):
    import concourse.bass as bass
    import concourse.mybir as mybir
    import concourse.tile as tile
    from concourse import bacc

    dt = mybir.dt
    f32, bf16, i32 = dt.float32, dt.bfloat16, dt.int32
    f32r = dt.float32r
    f16 = dt.float16
    AF = mybir.ActivationFunctionType
    OP = mybir.AluOpType
    AX = mybir.AxisListType

    nc = bacc.Bacc("TRN2", target_bir_lowering=False, debug=False)

    def din(name, shape, dtp=f32):
        return nc.dram_tensor(name, shape, dtp, kind="ExternalInput").ap()

    xqT = din("xqT", [E, NL], f16)
    xkT = din("xkT", [E, N], f16)
    xvT = din("xvT", [E, N], f16)
    adjm = din("adjm", [128, MASKW], f16)   # host-packed chunk-layout mask
    WqT = din("WqT", [E, E], f16)           # pre-scaled by 1/8 on host
    WkT = din("WkT", [E, KVE], f16)
    WvT = din("WvT", [E, KVE], f16)
    WoT = din("WoT", [E, E], f16)
    bq2d = din("bq2d", [EC, 128])           # bq/8
    bk2d = din("bk2d", [2, 128])
    bv2d = din("bv2d", [2, 128])
    bo1 = din("bo1", [1, E], f16)
    lng = din("lng", [EC, 128])
    lnb = din("lnb", [EC, 128])
    ones1h = din("ones1h", [1, 128], f16)
    y = nc.dram_tensor("y", [NL, E], f32, kind="ExternalOutput").ap()

    with tile.TileContext(nc) as tc, nc.allow_low_precision(
            "f16 operands for PE fast-path matmuls are intentional"):
        with (
            tc.tile_pool(name="const", bufs=1) as pc,
            tc.tile_pool(name="persist", bufs=1) as pp,
        ):
            ident = pc.tile([128, 128], f16, tag="ident")
            from concourse.masks import make_identity
            make_identity(nc, ident[:])
            ones_m1 = pc.tile([128, 1], f16, tag="ones_m1")
            nc.sync.dma_start(ones_m1[:], ones1h)
            ones_k1h = pc.tile([1, 128], f16, tag="ones_k1h")
            nc.sync.dma_start(ones_k1h[:], ones1h)
            eps_c = pc.tile([1, 1], f32, tag="eps_c")
            nc.gpsimd.memset(eps_c[:], LN_EPS)

            # persistent tiles (out-proj/LN constants: DMAs emitted late, after
            # phase 2, so they don't block the queue ahead of phase-1 inputs)
            wo_sb = [pp.tile([128, E], f16, tag=f"wo{e}", name=f"wo{e}") for e in range(EC)]
            bo_sb = pp.tile([1, E], f16, tag="bo", name="bo")
            lng_sb = [pp.tile([128, 1], f32, tag=f"lng{e}", name=f"lng{e}") for e in range(EC)]
            lnb_sb = [pp.tile([128, 1], f32, tag=f"lnb{e}", name=f"lnb{e}") for e in range(EC)]

            kT_sb = [pp.tile([128, N], f16, tag=f"kt{m}", name=f"kt{m}") for m in range(2)]
            # v_ext[k]: [128 s, 4 heads x 65] = per head 64 v dims + ones col
            v_ext = [pp.tile([128, 4 * 65], f16, tag=f"vx{k}", name=f"vx{k}") for k in range(SB)]
            meanv = [pp.tile([128, 1], f32, tag=f"mv{m}", name=f"mv{m}") for m in range(2)]
            mv2 = [pp.tile([128, 1], f32, tag=f"mv2{h}", name=f"mv2{h}") for h in range(HK)]
            # per-head q tiles, zero-padded to the full 128 partitions: head g
            # (kv head h = g//4) holds data at partitions [(h%2)*64, +64) --
            # matching h's partition half inside kT_sb[h//2] -- and ZEROS in
            # the other half. The sim matmul then contracts over all 128
            # partitions of kT_sb (both kv heads), with the zero half of q
            # nulling the wrong head's contribution. K=128 keeps the PE in
            # full-array mode at full streaming rate (a solo K=64 stream is
            # hardware-limited to half rate).
            qz = [pp.tile([128, NL], f16, tag=f"qz{g}", name=f"qz{g}") for g in range(HQ)]
            for g in range(HQ):
                pad = (1 - (g // G) % 2) * 64
                nc.gpsimd.memset(qz[g][pad:pad + 64, :], 0.0)
            attT = [pp.tile([128, NL], f16, tag=f"at{e}", name=f"at{e}") for e in range(EC)]
            mk_all = pp.tile([128, MASKW], f16, tag="mk", name="mk")
            if with_em:
                em_row = pp.tile([1, NL], f32, tag="em_row", name="em_row")
                em_bc = pp.tile([128, NL], f32, tag="em_bc", name="em_bc")

            # ---------------- phase 1: q projection ----------------
            with (
                tc.tile_pool(name="wq", bufs=1) as pwq,
                tc.tile_pool(name="xq", bufs=1) as pxq,
                tc.tile_pool(name="bq", bufs=1) as pbq,
                tc.tile_pool(name="psq", bufs=2, space="PSUM") as psq,
            ):
                wq_sb = [pwq.tile([128, E], f16, tag=f"wq{e}", name=f"wq{e}") for e in range(EC)]
                xq_sb = [pxq.tile([128, NL], f16, tag=f"xq{e}", name=f"xq{e}") for e in range(EC)]
                bq_sb = [pbq.tile([128, 1], f32, tag=f"bq{m}", name=f"bq{m}") for m in range(EC)]
                for e in range(EC):
                    nc.sync.dma_start(wq_sb[e][:], WqT[e * 128:(e + 1) * 128, :])
                    nc.sync.dma_start(xq_sb[e][:], xqT[e * 128:(e + 1) * 128, :])
                    nc.sync.dma_start(bq_sb[e][:], bq2d[e:e + 1, :])
                nc.sync.dma_start(mk_all[:], adjm)
                for mt2 in range(EC // 2):
                    psA = psq.tile([128, NL], f32, tag="psqA", name="psqA")
                    psB = psq.tile([128, NL], f32, tag="psqB", name="psqB")
                    for e in range(EC):
                        for mt, ps in ((2 * mt2, psA), (2 * mt2 + 1, psB)):
                            nc.tensor.matmul(
                                ps[:], wq_sb[e][:, mt * 128:(mt + 1) * 128],
                                xq_sb[e][:], start=(e == 0), stop=(e == EC - 1))
                    for mt, ps in ((2 * mt2, psA), (2 * mt2 + 1, psB)):
                     for t in range(2):
                        g = 2 * mt + t
                        slot = (g // G) % 2
                        nc.scalar.activation(
                            qz[g][slot * 64:(slot + 1) * 64, :],
                            ps[t * 64:(t + 1) * 64, :], AF.Identity,
                            bias=bq_sb[mt][t * 64:(t + 1) * 64, :], scale=1.0)

            # ---------------- phase 2: k/v projections ----------------
            with (
                tc.tile_pool(name="wkv", bufs=1) as pwkv,
                tc.tile_pool(name="xkv", bufs=3) as pxkv,
                tc.tile_pool(name="vt", bufs=2) as pvt,
                tc.tile_pool(name="vs", bufs=1) as pvs,
                tc.tile_pool(name="pskv", bufs=2, space="PSUM") as pskv,
                tc.tile_pool(name="pst", bufs=2, space="PSUM") as pst,
            ):
                wk_sb = [pwkv.tile([128, KVE], f16, tag=f"wk{e}", name=f"wk{e}") for e in range(EC)]
                wv_sb = [pwkv.tile([128, KVE], f16, tag=f"wv{e}", name=f"wv{e}") for e in range(EC)]
                bk_sb = [pwkv.tile([128, 1], f32, tag=f"bk{m}", name=f"bk{m}") for m in range(2)]
                bv_sb = [pwkv.tile([128, 1], f32, tag=f"bv{m}", name=f"bv{m}") for m in range(2)]
                for e in range(EC):
                    nc.sync.dma_start(wk_sb[e][:], WkT[e * 128:(e + 1) * 128, :])
                    nc.sync.dma_start(wv_sb[e][:], WvT[e * 128:(e + 1) * 128, :])
                for m in range(2):
                    nc.sync.dma_start(bk_sb[m][:], bk2d[m:m + 1, :])
                    nc.sync.dma_start(bv_sb[m][:], bv2d[m:m + 1, :])
                vsum = [pvs.tile([128, 4], f32, tag=f"vsum{m}", name=f"vsum{m}") for m in range(2)]
                for st in (0, 3, 1, 2):  # chunk 0 needs s-tiles 0 and 3 first
                    sl = slice(st * 512, (st + 1) * 512)
                    xk_sb = [pxkv.tile([128, 512], f16, tag=f"xk{e % 2}", name=f"xk{e}") for e in range(EC)]
                    xv_sb = [pxkv.tile([128, 512], f16, tag=f"xv{e % 2}", name=f"xv{e}") for e in range(EC)]
                    for e in range(EC):
                        nc.sync.dma_start(xk_sb[e][:], xkT[e * 128:(e + 1) * 128, sl])
                        nc.sync.dma_start(xv_sb[e][:], xvT[e * 128:(e + 1) * 128, sl])
                    for mt in range(2):
                        psk = pskv.tile([128, 512], f32, tag="psk")
                        psv = pskv.tile([128, 512], f32, tag="psv")
                        for e in range(EC):
                            nc.tensor.matmul(
                                psk[:], wk_sb[e][:, mt * 128:(mt + 1) * 128],
                                xk_sb[e][:], start=(e == 0), stop=(e == EC - 1))
                            nc.tensor.matmul(
                                psv[:], wv_sb[e][:, mt * 128:(mt + 1) * 128],
                                xv_sb[e][:], start=(e == 0), stop=(e == EC - 1))
                        nc.scalar.activation(kT_sb[mt][:, sl], psk[:], AF.Identity,
                                             bias=bk_sb[mt][:], scale=1.0)
                        vt = pvt.tile([128, 512], f16, tag="vt")
                        nc.scalar.activation(vt[:], psv[:], AF.Identity,
                                             bias=bv_sb[mt][:], scale=1.0)
                        nc.vector.reduce_sum(vsum[mt][:, st:st + 1], vt[:], axis=AX.X)
                        for ss in range(4):
                            k = st * 4 + ss
                            pt = pst.tile([128, 128], f16, tag="pt")
                            nc.tensor.transpose(pt[:], vt[:, ss * 128:(ss + 1) * 128],
                                                ident[:])
                            src = pt[:].rearrange("p (h x) -> p h x", h=2)
                            dst = v_ext[k][:].rearrange("p (h x) -> p h x", h=4)
                            nc.vector.tensor_copy(dst[:, 2 * mt:2 * mt + 2, 0:64], src)
                for k in range(SB):
                    one_col = v_ext[k][:].rearrange("p (h x) -> p h x", h=4)[:, :, 64:65]
                    nc.gpsimd.memset(one_col, 1.0)
                for m in range(2):
                    nc.vector.tensor_reduce(meanv[m][:], vsum[m][:], axis=AX.X,
                                            op=OP.add)
                    nc.vector.tensor_scalar_mul(meanv[m][:], meanv[m][:], 1.0 / N)
                for h in range(HK):
                    src = meanv[h // 2][(h % 2) * 64:(h % 2) * 64 + 64, :]
                    nc.vector.tensor_copy(mv2[h][0:64, :], src)
                    nc.vector.tensor_copy(mv2[h][64:128, :], src)

            # late DMAs: out-proj/LN constants, needed only in phase 4
            for e in range(EC):
                nc.sync.dma_start(wo_sb[e][:], WoT[e * 128:(e + 1) * 128, :])
                nc.sync.dma_start(lng_sb[e][:], lng[e:e + 1, :])
                nc.sync.dma_start(lnb_sb[e][:], lnb[e:e + 1, :])
            nc.sync.dma_start(bo_sb[:], bo1)

            # ---------------- phase 3: attention ----------------
            # empty (fully-masked) rows can only be tiny global row indices
            # (P ~ 2^-(r+1)); they land in local columns < EMW, so the
            # empty-row fallback ops are restricted to that column range.
            EMW = NL
            with (
                tc.tile_pool(name="exs", bufs=3) as pex,
                tc.tile_pool(name="nrm", bufs=2) as pnrm,
                tc.tile_pool(name="pssim", bufs=1, space="PSUM") as psim,
                tc.tile_pool(name="pspv", bufs=1, space="PSUM") as ppv,
            ):
                first = True
                for pair in range(2):
                    for gi in range(G):
                        heads = [(2 * pair, 0), (2 * pair + 1, 1)]
                        pvs_t = {}
                        for h, slot in heads:
                            pvs_t[slot] = ppv.tile([128, 512], f32, tag=f"pv{slot}",
                                                   name=f"pv{slot}")

                        def emit_pv(blocks, exs):
                            for h, slot in heads:
                                ex = exs[slot]
                                for k, off in blocks:
                                    c = _cols(k)
                                    nc.tensor.matmul(
                                        pvs_t[slot][0:65, 32 * k:512],
                                        v_ext[k][:, 65 * h:65 * h + 65],
                                        ex[:, off:off + c],
                                        start=(k == 0), stop=(k == 8),
                                        skip_group_check=True)

                        pend = []
                        for ci, blocks in enumerate(CH):
                            W = CHW[ci]
                            exs = {}
                            sims = {}
                            for h, slot in heads:
                                sims[slot] = psim.tile([128, 1536], f32,
                                                       tag=f"sim{slot}",
                                                       name=f"sim{slot}")
                            for h, slot in heads:
                                g = h * G + gi
                                kh = kT_sb[h // 2]
                                for k, off in blocks:
                                    c = _cols(k)
                                    nc.tensor.matmul(
                                        sims[slot][:, off:off + c],
                                        kh[:, k * 128:(k + 1) * 128],
                                        qz[g][:, 32 * k:512],
                                        start=True, stop=True)
                            for h, slot in heads:
                                ex = pex.tile([128, 1536], f16, tag=f"ex{slot}",
                                              name=f"ex{slot}")
                                nc.scalar.activation(ex[:, 0:W], sims[slot][:, 0:W],
                                                     AF.Exp)
                                nc.vector.tensor_tensor(
                                    ex[:, 0:W], ex[:, 0:W],
                                    mk_all[:, CHBASE[ci]:CHBASE[ci] + W], op=OP.mult)
                                exs[slot] = ex
                            pend.append((blocks, exs))
                            if len(pend) > PIPE:
                                emit_pv(*pend.pop(0))
                        while pend:
                            emit_pv(*pend.pop(0))
                        for h, slot in heads:
                            g = h * G + gi
                            pv = pvs_t[slot]
                            if first and with_em:
                                # row-emptiness is head-independent
                                nc.vector.tensor_scalar(em_row[:, 0:EMW],
                                                        pv[64:65, 0:EMW], 0.0,
                                                        None, op0=OP.is_equal)
                                nc.gpsimd.partition_broadcast(em_bc[:, 0:EMW],
                                                              em_row[:, 0:EMW])
                                first = False
                            den = pnrm.tile([1, NL], f32, tag=f"den{slot}",
                                            name=f"den{slot}")
                            if with_em:
                                nc.vector.tensor_tensor(den[:, 0:EMW],
                                                        pv[64:65, 0:EMW],
                                                        em_row[:, 0:EMW], op=OP.add)
                                nc.vector.reciprocal_approx_fast(
                                    den[:, 0:EMW], den[:, 0:EMW])
                                if EMW < NL:
                                    nc.vector.reciprocal_approx_fast(
                                        den[:, EMW:], pv[64:65, EMW:])
                            else:
                                nc.vector.reciprocal_approx_fast(den[:],
                                                                 pv[64:65, :])
                            recb = pnrm.tile([128, NL], f32, tag=f"recb{slot}",
                                             name=f"recb{slot}")
                            nc.gpsimd.partition_broadcast(recb[:], den[:])
                            p0 = (g % 2) * 64
                            att = attT[g // 2][p0:p0 + 64, :]
                            nc.vector.tensor_tensor(att, pv[0:64, :], recb[0:64, :],
                                                    op=OP.mult)
                            if with_em:
                                nc.vector.scalar_tensor_tensor(
                                    att[:, 0:EMW], em_bc[p0:p0 + 64, 0:EMW],
                                    mv2[h][p0:p0 + 64, :], att[:, 0:EMW],
                                    op0=OP.mult, op1=OP.add)

            # ---------------- phase 4: layernorm + out projection ----------------
            with (
                tc.tile_pool(name="lnt", bufs=2) as plnt,
                tc.tile_pool(name="ysb", bufs=2) as pysb,
                tc.tile_pool(name="psst", bufs=1, space="PSUM") as psst,
                tc.tile_pool(name="psy", bufs=2, space="PSUM") as psy,
            ):
                st_sum = psst.tile([1, NL], f32, tag="ssum")
                st_sq = psst.tile([1, NL], f32, tag="ssq")
                for e in range(EC):
                    nc.tensor.matmul(st_sum[:], ones_m1[:], attT[e][:],
                                     start=(e == 0), stop=(e == EC - 1))
                    sq = plnt.tile([128, NL], f16, tag="sq")
                    nc.scalar.activation(sq[:], attT[e][:], AF.Square)
                    nc.tensor.matmul(st_sq[:], ones_m1[:], sq[:],
                                     start=(e == 0), stop=(e == EC - 1))
                mu = plnt.tile([1, NL], f32, tag="mu")
                nc.vector.tensor_scalar_mul(mu[:], st_sum[:], 1.0 / E)
                var = plnt.tile([1, NL], f32, tag="var")
                nc.vector.tensor_scalar_mul(var[:], st_sq[:], 1.0 / E)
                mu2 = plnt.tile([1, NL], f32, tag="mu2")
                nc.vector.tensor_tensor(mu2[:], mu[:], mu[:], op=OP.mult)
                nc.vector.tensor_tensor(var[:], var[:], mu2[:], op=OP.subtract)
                sd = plnt.tile([1, NL], f32, tag="sd")
                nc.scalar.activation(sd[:], var[:], AF.Sqrt, bias=eps_c[:])
                nc.vector.reciprocal(sd[:], sd[:])
                mu_h = plnt.tile([1, NL], f16, tag="mu_h")
                nc.vector.tensor_copy(mu_h[:], mu[:])
                sd_h = plnt.tile([1, NL], f16, tag="sd_h")
                nc.vector.tensor_copy(sd_h[:], sd[:])
                mbb = plnt.tile([128, NL], f16, tag="mbb")
                nc.gpsimd.partition_broadcast(mbb[:], mu_h[:])
                ibb = plnt.tile([128, NL], f16, tag="ibb")
                nc.gpsimd.partition_broadcast(ibb[:], sd_h[:])
                for e in range(EC):
                    nc.vector.tensor_tensor(attT[e][:], attT[e][:], mbb[:],
                                            op=OP.subtract)
                    nc.vector.tensor_tensor(attT[e][:], attT[e][:], ibb[:],
                                            op=OP.mult)
                    nc.vector.tensor_scalar(attT[e][:], attT[e][:], lng_sb[e][:],
                                            lnb_sb[e][:], op0=OP.mult, op1=OP.add)
                for nt in range(4):
                    pyA = psy.tile([128, 512], f32, tag="pyA", name="pyA")
                    pyB = psy.tile([128, 512], f32, tag="pyB", name="pyB")
                    pys = (pyA, pyB)
                    for e in range(EC):
                        for oc in range(2):
                            nc.tensor.matmul(
                                pys[oc][:], attT[e][:, nt * 128:(nt + 1) * 128],
                                wo_sb[e][:, oc * 512:(oc + 1) * 512],
                                start=(e == 0), stop=False)
                    for oc in range(2):
                        nc.tensor.matmul(pys[oc][:], ones_k1h[:],
                                         bo_sb[0:1, oc * 512:(oc + 1) * 512],
                                         start=False, stop=True)
                        ys = pysb.tile([128, 512], f32, tag="ys")
                        nc.scalar.activation(ys[:], pys[oc][:], AF.Identity)
                        nc.sync.dma_start(
                            y[nt * 128:(nt + 1) * 128, oc * 512:(oc + 1) * 512],
                            ys[:])
    nc.finalize()
    return nc


def shard_inputs(inputs):
    q = np.asarray(inputs["query"], np.float32)
    k = np.asarray(inputs["key"], np.float32)
    v = np.asarray(inputs["value"], np.float32)
    adj = np.asarray(inputs["adj"], np.int32)
    WqT8 = (np.ascontiguousarray(np.asarray(inputs["Wq"], np.float32).T)
            / np.float32(8.0)).astype(np.float16)
    WkT = np.ascontiguousarray(np.asarray(inputs["Wk"], np.float32).T).astype(np.float16)
    WvT = np.ascontiguousarray(np.asarray(inputs["Wv"], np.float32).T).astype(np.float16)
    WoT = np.ascontiguousarray(np.asarray(inputs["Wo"], np.float32).T).astype(np.float16)
    bq8 = (np.asarray(inputs["bq"], np.float32) / np.float32(8.0)).reshape(EC, 128)
    bk2 = np.asarray(inputs["bk"], np.float32).reshape(2, 128)
    bv2 = np.asarray(inputs["bv"], np.float32).reshape(2, 128)
    bo1 = np.asarray(inputs["bo"], np.float32).reshape(1, E).astype(np.float16)
    lng = np.asarray(inputs["ln_g"], np.float32).reshape(EC, 128)
    lnb = np.asarray(inputs["ln_b"], np.float32).reshape(EC, 128)

    shared = dict(WqT=WqT8, WkT=WkT, WvT=WvT, WoT=WoT, bq2d=bq8, bk2d=bk2,
                  bv2d=bv2, bo1=bo1, lng=lng, lnb=lnb,
                  ones1h=np.ones((1, 128), np.float16))
    per_b = []
    s_idx = np.arange(N)
    for b in range(B):
        per_b.append((np.ascontiguousarray(k[b].T).astype(np.float16),
                      np.ascontiguousarray(v[b].T).astype(np.float16)))
    in_maps = []
    for c in range(8):
        b, j = divmod(c, 4)
        rows = np.arange(j, N, 4)
        causal = s_idx[None, :] <= rows[:, None]          # [NL, N]
        adjc = np.where(causal, adj[b][rows], 0)          # [NL rows, N s]
        mT = adjc.T.astype(np.float16)                    # [N s, NL]
        adjmp = np.zeros((128, MASKW), np.float16)
        for ci, blocks in enumerate(CH):
            base = CHBASE[ci]
            for kk, off in blocks:
                c_ = 512 - 32 * kk
                adjmp[:, base + off:base + off + c_] = \
                    mT[128 * kk:128 * (kk + 1), 32 * kk:512]
        m = dict(shared)
        m["xqT"] = np.ascontiguousarray(q[b][rows].T).astype(np.float16)
        m["xkT"], m["xvT"] = per_b[b]
        m["adjm"] = adjmp
        in_maps.append(m)
    return in_maps


def unshard_outputs(results):
    out = np.empty((B, N, E), np.float32)
    for c in range(8):
        b, j = divmod(c, 4)
        out[b, j::4, :] = results[c]["y"]
    return out


def kernel(**inputs):
    from concourse.bass_utils import run_bass_kernel_spmd

    if "nc" not in _PROG_CACHE:
        _PROG_CACHE["nc"] = build_program()
    nc = _PROG_CACHE["nc"]
    in_maps = shard_inputs(inputs)
    res = run_bass_kernel_spmd(nc, in_maps, core_ids=list(range(8)))
    return unshard_outputs(res.results)
